# revision 17
# baseline (speedup 1.0000x reference)
"""Trainium2 Bass kernel for nn_BondLenConstrain.

Contract: kernel(**inputs) takes the FULL (unsharded) inputs of
reference.setup_inputs() and returns the full [64, 4, 2048, 2] float32
resiEnergy tensor.  Data-parallel over the batch axis across 8 NeuronCores
(8 batches per core).

Host (numpy, indexing only): scatter atoms into dense residue grids exactly
like the reference, build the `todo` mask, gather the tiny per-residue-type
tables into per-residue coefficient planes (masked pairs get all-zero
coefficients -> device formula returns exactly 0), and broadcast the
(identical) nalt lanes of the output on assembly.

Device math per residue pair (r-1, r), with P=C_{r-1}, Q=N_r, A=CA_r,
B=CA_{r-1}:
    v2 = A-Q, v1 = P-Q, v3 = B-P        (planar [plane][R] layout)
    d22=|v2|^2, d11=|v1|^2, d33=|v3|^2 ; c1 = v1.v2, c2 = v3.v1
    M = d11*d22 (resp. d11*d33), s = sqrt(M - c^2)
    half-angle identity:  angle(v1,v2) = pi/2 - 2*arctan(c1/(s1+sqrt(M1)))
      (argument in [-1,1] automatically; arctan odd -> no sign handling;
       hardware Arctan table domain is ~[-pi/2,pi/2])
    f1 = sqrt(d11)
    U_d = fb_d * B_d - A_d   with fb = [f1, phi1, phi2] and host-baked A,B
    score_d = min(U_d^2, C_d); e = sum_d score_d
A/B/C fold mean/std/weight/todo (masked pairs: A=B=C=0 -> e=0).

Perf structure (v3):
  * input DMAs chained X0 -> P0 -> X1 -> P1 so chunk0's coords get full
    DMA bandwidth instead of fair-sharing with 3 other transfers
  * 1/x via the single-instruction custom-DVE reciprocal_approx_fast
    (nc.vector.reciprocal measured 4us per 512 elems)
  * scoring tail (P coeffs, T, FB, U, Z, ZC) in fp16: DVE 2x_1p mode +
    half the P DMA bytes; rel err ~2.8e-3 on the grading data (gate 2e-2)
  * Square/Sqrt in one act table, Arctan/Square in another; phase A (both
    chunks through sqrt) emitted before phase B under tile_wait_until so
    the Tile scheduler keeps 2 ACT_TABLE_LOADs total
"""

import os
import numpy as np

PAD = -999.0
PAD_I = -999
NB, MC, MR = 64, 4, 2048
NALT = 2
NCORES = 8
BPC = NB // NCORES            # batches per core
CH = int(os.environ.get("BLC_CHUNKS", "2"))  # pipeline chunks per core
KC = 4 * CH                   # blocks per (batch, chain) across full chain
R = MR // KC                  # residues (pairs) per partition
S = R + 1                     # slots per atom plane (halo)
EPS = 1e-12
CL = 1.0 / (EPS * np.sqrt(np.pi))

_PROGRAM_CACHE = {}
LAST_RESULT = None            # BassKernelResults of the last run (for test.py)
TRACE = bool(int(os.environ.get("BLC_TRACE", "0")))


def _build_program():
    import concourse.bass as bass
    import concourse.tile as tile
    from concourse import bacc, mybir
    from concourse.bass import _add_dep_helper

    dt = mybir.dt.float32
    hf = mybir.dt.float16
    Alu = mybir.AluOpType
    Act = mybir.ActivationFunctionType

    nc = bacc.Bacc("TRN2", target_bir_lowering=False, debug=False)

    G_t = nc.declare_dram_parameter("g", [BPC, MC, KC, 9 * S], dt,
                                    isOutput=False)
    P_t = nc.declare_dram_parameter("pr", [BPC, MC, KC, 9 * R], hf,
                                    isOutput=False)
    O_t = nc.declare_dram_parameter("out", [BPC, MC, MR], dt, isOutput=True)

    bc = BPC // CH            # batches per chunk
    bufs = min(CH, 2)

    with tile.TileContext(nc) as tc:
        with (
            tc.tile_pool(name="px", bufs=bufs) as px,
            tc.tile_pool(name="pp", bufs=bufs) as pp,
            tc.tile_pool(name="ps", bufs=bufs) as ps,
        ):
            loads = []
            prev_dma = None
            for c in range(CH):
                b0 = c * bc
                X = px.tile([128, 9 * S], dt, tag="x")
                P = pp.tile([128, 9 * R], hf, tag="p")
                dx = nc.sync.dma_start(X[:], G_t[b0:b0 + bc])
                if prev_dma is not None:
                    _add_dep_helper(dx.ins, prev_dma.ins, sync=True,
                                    reason="serialize input DMAs")
                dp = nc.sync.dma_start(P[:], P_t[b0:b0 + bc])
                _add_dep_helper(dp.ins, dx.ins, sync=True,
                                reason="serialize input DMAs")
                prev_dma = dp
                loads.append((X, P))

            # dummy Sqrt so the act-table pass picks the sqrt set for its
            # initial load (otherwise the first Square binds to another set
            # and Sqrt forces a second mid-stream load)
            dum = ps.tile([128, 1], dt, tag="dum")
            nc.gpsimd.memset(dum[:], 1.0)
            nc.scalar.activation(dum[:], dum[:], Act.Sqrt)

            mids = []
            # ---- phase A per chunk: geometry through sqrt (sqrt table) ----
            for c in range(CH):
                X, P = loads[c]
                Xv = X[:].rearrange("p (a c s) -> p a c s", a=3, c=3)
                V = px.tile([128, 9 * R], dt, tag="v")
                Vv = V[:].rearrange("p (v c l) -> p v c l", v=3, c=3)
                # planes: v2 = CA_next - N_next ; v1 = C_prev - N_next ;
                # v3 = CA_prev - C_prev   (atom order in G: N, CA, C)
                nc.vector.tensor_sub(Vv[:, 0], Xv[:, 1, :, 1:S],
                                     Xv[:, 0, :, 1:S])
                nc.vector.tensor_sub(Vv[:, 1], Xv[:, 2, :, 0:R],
                                     Xv[:, 0, :, 1:S])
                nc.vector.tensor_sub(Vv[:, 2], Xv[:, 1, :, 0:R],
                                     Xv[:, 2, :, 0:R])

                # W = [v2^2 | v1^2 | v3^2 | v1*v2 | v3*v1] as 5 groups of
                # 3 xyz planes; one pair of strided adds then contracts all
                # five dot products at once.
                W = px.tile([128, 15 * R], dt, tag="w")
                nc.scalar.activation(W[:, 0:9 * R], V[:], Act.Square)
                nc.vector.tensor_mul(W[:, 9 * R:15 * R], V[:, 3 * R:9 * R],
                                     V[:, 0:6 * R])
                Wv = W[:].rearrange("p (g c l) -> p g c l", g=5, c=3)
                DC = ps.tile([128, 5 * R], dt, tag="dc")
                # [d22 | d11 | d33 | c1 | c2]
                DCv = DC[:].rearrange("p (g l) -> p g l", g=5)
                nc.vector.tensor_add(DCv, Wv[:, :, 0], Wv[:, :, 1])
                nc.vector.tensor_add(DCv, DCv, Wv[:, :, 2])

                SQI = ps.tile([128, 4 * R], dt, tag="sqi")  # [s^2 2R | M 2R]
                nc.vector.tensor_mul(SQI[:, 2 * R:3 * R], DC[:, R:2 * R],
                                     DC[:, 0:R])
                nc.vector.tensor_mul(SQI[:, 3 * R:4 * R], DC[:, R:2 * R],
                                     DC[:, 2 * R:3 * R])
                CSQ = ps.tile([128, 2 * R], dt, tag="csq")
                nc.scalar.activation(CSQ[:], DC[:, 3 * R:5 * R], Act.Square)
                nc.vector.tensor_sub(SQI[:, 0:2 * R], SQI[:, 2 * R:4 * R],
                                     CSQ[:])
                nc.vector.tensor_scalar_max(SQI[:], SQI[:], 1e-30)
                SRT = ps.tile([128, 4 * R], dt, tag="srt")  # [s 2R | rtM 2R]
                nc.scalar.activation(SRT[:], SQI[:], Act.Sqrt)

                FB = ps.tile([128, 3 * R], hf, tag="fb")   # [f1 | phi1 | phi2]
                nc.scalar.activation(FB[:, 0:R], DC[:, R:2 * R], Act.Sqrt)

                # den = s + sqrt(M), overwriting s (not needed afterwards)
                nc.vector.tensor_add(SRT[:, 0:2 * R], SRT[:, 0:2 * R],
                                     SRT[:, 2 * R:4 * R])
                REC = ps.tile([128, 2 * R], dt, tag="rec")
                nc.vector.reciprocal_approx_fast(out=REC[:],
                                                 in_=SRT[:, 0:2 * R])
                T = ps.tile([128, 2 * R], hf, tag="t")
                nc.vector.tensor_mul(T[:], DC[:, 3 * R:5 * R], REC[:])
                mids.append((P, T, FB))

            # ---- phase B per chunk: arctan + scoring (trig table) ---------
            with tc.tile_wait_until(1.0):
                for c in range(CH):
                    b0 = c * bc
                    P, T, FB = mids[c]
                    nc.scalar.activation(FB[:, R:3 * R], T[:], Act.Arctan)
                    U = ps.tile([128, 3 * R], hf, tag="u")
                    nc.vector.tensor_mul(U[:], FB[:], P[:, 3 * R:6 * R])
                    nc.vector.tensor_sub(U[:], U[:], P[:, 0:3 * R])
                    nc.scalar.activation(U[:], U[:], Act.Square)
                    nc.vector.tensor_tensor(U[:], U[:], P[:, 6 * R:9 * R],
                                            op=Alu.min)
                    E = ps.tile([128, R], dt, tag="e")
                    nc.vector.tensor_add(E[:], U[:, 0:R], U[:, R:2 * R])
                    nc.vector.tensor_add(E[:], E[:], U[:, 2 * R:3 * R])
                    nc.sync.dma_start(
                        O_t[b0:b0 + bc].rearrange("b c (k l) -> b c k l",
                                                  k=KC),
                        E[:])

    return nc


def _get_program():
    if "nc" not in _PROGRAM_CACHE:
        nc = _build_program()
        nc.finalize()   # Bacc: register allocation / DCE / wait legalization
        _PROGRAM_CACHE["nc"] = nc
    return _PROGRAM_CACHE["nc"]


def _host_prep(atom_description, coords, mean, std, weight):
    ad = np.asarray(atom_description)
    coords = np.asarray(coords, dtype=np.float32)
    b, ch, rs, rn, an = (ad[:, i] for i in range(5))
    valid = (b >= 0) & (b < NB) & (ch >= 0) & (ch < MC) & (rs >= 0) & (rs < MR)

    def scat3(mask):
        A = np.full((NB, MC, MR, 3), PAD, np.float32)
        m = mask & valid
        A[b[m], ch[m], rs[m]] = coords[m]
        return A

    Narr, CAarr, Carr = scat3(an == 0), scat3(an == 1), scat3(an == 2)
    seq = np.full((NB, MC, MR), PAD_I, np.int64)
    m = (an == 1) & valid
    seq[b[m], ch[m], rs[m]] = rn[m]

    todo = ((Narr[:, :, 1:, 0] != PAD) & (Carr[:, :, :-1, 0] != PAD)
            & (CAarr[:, :, 1:, 0] != PAD) & (CAarr[:, :, :-1, 0] != PAD)
            & (seq[:, :, 1:] != PAD_I) & (seq[:, :, :-1] != PAD_I))
    sidx = np.clip(np.where(todo, seq[:, :, 1:], 0), 0, 19)

    w0 = float(np.asarray(weight).reshape(-1)[0])
    s_w = 1.0 - np.tanh(-w0)
    sq = np.sqrt(s_w)
    mu = np.asarray(mean, np.float64)
    sd = np.asarray(std, np.float64)
    q = 1.0 / (sd * np.sqrt(2.0))
    qs = q * sq
    # A = subtractand, B = multiplier for fb=[f1, phi1, phi2], C = clamp.
    # theta1 = pi/2 - 2*phi1 ; theta2 = pi/2 + 2*phi2  (reference's second
    # angle uses N_next-C_prev = -v1; arctan's oddness folds the sign into
    # B2 = -2*q2).
    tab = np.empty((20, 9))
    tab[:, 0] = mu[:, 0] * qs[:, 0]
    tab[:, 1] = (np.pi / 2 - mu[:, 1]) * qs[:, 1]
    tab[:, 2] = (np.pi / 2 - mu[:, 2]) * qs[:, 2]
    tab[:, 3] = qs[:, 0]
    tab[:, 4] = 2.0 * qs[:, 1]
    tab[:, 5] = -2.0 * qs[:, 2]
    tab[:, 6:9] = s_w * np.maximum(np.log(CL * q), 0.0)
    tab = tab.astype(np.float32)

    params = np.zeros((NB, MC, MR, 9), np.float32)
    params[:, :, 1:, :] = tab[sidx] * todo[..., None].astype(np.float32)
    # P row layout per (b,c,k): planar [A0|A1|A2|B0|B1|B2|C0|C1|C2] planes
    # of R, fp16.
    pb = params.reshape(NB, MC, KC, R, 9)
    pblk = np.ascontiguousarray(
        pb.transpose(0, 1, 2, 4, 3)).reshape(NB, MC, KC, 9 * R)
    pblk = pblk.astype(np.float16)

    # G row: planar [atom(N,CA,C)][xyz][slot 0..R]; slot s of block k holds
    # residue k*R + s - 1; content 0.0 where that residue index is < 0.
    G = np.zeros((NB, MC, MR + 1, 3, 3), np.float32)
    G[:, :, 1:, 0] = Narr
    G[:, :, 1:, 1] = CAarr
    G[:, :, 1:, 2] = Carr
    GB = np.empty((NB, MC, KC, 3, 3, S), np.float32)
    for k in range(KC):
        # [b, c, slot, atom, xyz] -> [b, c, atom, xyz, slot]
        GB[:, :, k] = G[:, :, k * R:k * R + S].transpose(0, 1, 3, 4, 2)
    return GB.reshape(NB, MC, KC, 9 * S), pblk


def _install_ntff_hook():
    """The agent image's antenv lacks axon_hooks; synthesize it so
    trace=True can reach the terminal's NRT profiler (dev-only path)."""
    import sys, types
    if "antenv.axon_hooks" in sys.modules:
        return True
    try:
        import antenv
        mod = types.ModuleType("antenv.axon_hooks")
        mod._hook = None

        def set_axon_ntff_profile_hook(h):
            mod._hook = h

        def get_axon_ntff_profile_hook():
            return mod._hook

        mod.set_axon_ntff_profile_hook = set_axon_ntff_profile_hook
        mod.get_axon_ntff_profile_hook = get_axon_ntff_profile_hook
        sys.modules["antenv.axon_hooks"] = mod
        antenv.axon_hooks = mod
        from trn_agent_boot.trn_boot import _ntff_profile_via_ctypes
        mod._hook = _ntff_profile_via_ctypes("/opt/axon/libaxon_pjrt.so")
        return True
    except Exception as e:  # pragma: no cover - profiling is best-effort
        print(f"ntff hook install failed: {e}")
        return False


def kernel(**inputs):
    global LAST_RESULT
    from concourse.bass_utils import run_bass_kernel_spmd
    if TRACE:
        _install_ntff_hook()

    G, pblk = _host_prep(
        inputs["atom_description"], inputs["coords"],
        inputs["mean"], inputs["std"], inputs["weight"])

    nc = _get_program()
    in_maps = [
        {"g": np.ascontiguousarray(G[i * BPC:(i + 1) * BPC]),
         "pr": np.ascontiguousarray(pblk[i * BPC:(i + 1) * BPC])}
        for i in range(NCORES)
    ]
    res = run_bass_kernel_spmd(nc, in_maps, list(range(NCORES)), trace=TRACE)
    LAST_RESULT = res
    e = np.concatenate([res.results[i]["out"] for i in range(NCORES)], axis=0)
    e = e.reshape(NB, MC, MR)
    out = np.repeat(e[..., None], NALT, axis=-1)
    return np.ascontiguousarray(out.astype(np.float32))


# revision 20
# speedup vs baseline: 1.3033x; 1.3033x over previous
"""Trainium2 Bass kernel for nn_BondLenConstrain.

Contract: kernel(**inputs) takes the FULL (unsharded) inputs of
reference.setup_inputs() and returns the full [64, 4, 2048, 2] float32
resiEnergy tensor.  Data-parallel over the batch axis across 8 NeuronCores
(8 batches per core).

Host (numpy, indexing only): scatter atoms into dense residue grids exactly
like the reference, build the `todo` mask, gather the tiny per-residue-type
tables into per-residue coefficient planes (masked pairs get all-zero
coefficients -> device formula returns exactly 0), and broadcast the
(identical) nalt lanes of the output on assembly.

Device math per residue pair (r-1, r), with P=C_{r-1}, Q=N_r, A=CA_r,
B=CA_{r-1}:
    v2 = A-Q, v1 = P-Q, v3 = B-P        (planar [plane][R] layout)
    d22=|v2|^2, d11=|v1|^2, d33=|v3|^2 ; c1 = v1.v2, c2 = v3.v1
    M = d11*d22 (resp. d11*d33), s = sqrt(M - c^2)
    half-angle identity:  angle(v1,v2) = pi/2 - 2*arctan(c1/(s1+sqrt(M1)))
      (argument in [-1,1] automatically; arctan odd -> no sign handling;
       hardware Arctan table domain is ~[-pi/2,pi/2])
    f1 = sqrt(d11)
    U_d = fb_d * B_d - A_d   with fb = [f1, phi1, phi2] and host-baked A,B
    score_d = min(U_d^2, C_d); e = sum_d score_d
A/B/C fold mean/std/weight/todo (masked pairs: A=B=C=0 -> e=0).

Perf structure (v3):
  * input DMAs chained X0 -> P0 -> X1 -> P1 so chunk0's coords get full
    DMA bandwidth instead of fair-sharing with 3 other transfers
  * 1/x via the single-instruction custom-DVE reciprocal_approx_fast
    (nc.vector.reciprocal measured 4us per 512 elems)
  * scoring tail (P coeffs, T, FB, U, Z, ZC) in fp16: DVE 2x_1p mode +
    half the P DMA bytes; rel err ~2.8e-3 on the grading data (gate 2e-2)
  * Square/Sqrt in one act table, Arctan/Square in another; phase A (both
    chunks through sqrt) emitted before phase B under tile_wait_until so
    the Tile scheduler keeps 2 ACT_TABLE_LOADs total
"""

import os
import numpy as np

PAD = -999.0
PAD_I = -999
NB, MC, MR = 64, 4, 2048
NALT = 2
NCORES = 8
BPC = NB // NCORES            # batches per core
CH = int(os.environ.get("BLC_CHUNKS", "2"))  # pipeline chunks per core
KC = 4 * CH                   # blocks per (batch, chain) across full chain
R = MR // KC                  # residues (pairs) per partition
S = R + 1                     # slots per atom plane (halo)
EPS = 1e-12
CL = 1.0 / (EPS * np.sqrt(np.pi))

_PROGRAM_CACHE = {}
LAST_RESULT = None            # BassKernelResults of the last run (for test.py)
TRACE = bool(int(os.environ.get("BLC_TRACE", "0")))


def _build_program():
    import concourse.bass as bass
    import concourse.tile as tile
    from concourse import bacc, mybir
    from concourse.bass import _add_dep_helper

    dt = mybir.dt.float32
    hf = mybir.dt.float16
    Alu = mybir.AluOpType
    Act = mybir.ActivationFunctionType

    nc = bacc.Bacc("TRN2", target_bir_lowering=False, debug=False)

    G_t = nc.declare_dram_parameter("g", [BPC, MC, KC, 9 * S], dt,
                                    isOutput=False)
    P_t = nc.declare_dram_parameter("pr", [BPC, MC, KC, 9 * R], hf,
                                    isOutput=False)
    O_t = nc.declare_dram_parameter("out", [BPC, MC, MR], dt, isOutput=True)

    bc = BPC // CH            # batches per chunk
    bufs = min(CH, 2)

    with tile.TileContext(nc) as tc:
        with (
            tc.tile_pool(name="px", bufs=bufs) as px,
            tc.tile_pool(name="pp", bufs=bufs) as pp,
            tc.tile_pool(name="ps", bufs=bufs) as ps,
        ):
            loads = []
            prev_dma = None
            for c in range(CH):
                b0 = c * bc
                # split coords into [N|CA] and [C] tiles: the first sub
                # (v2 = CA-N) can start 1/3 of a transfer earlier
                XA = px.tile([128, 6 * S], dt, tag="xa")
                XB = px.tile([128, 3 * S], dt, tag="xb")
                P = pp.tile([128, 9 * R], hf, tag="p")
                dxa = nc.sync.dma_start(XA[:], G_t[b0:b0 + bc, :, :, 0:6 * S])
                if prev_dma is not None:
                    _add_dep_helper(dxa.ins, prev_dma.ins, sync=True,
                                    reason="serialize input DMAs")
                dxb = nc.sync.dma_start(XB[:], G_t[b0:b0 + bc, :, :,
                                                   6 * S:9 * S])
                _add_dep_helper(dxb.ins, dxa.ins, sync=True,
                                reason="serialize input DMAs")
                dp = nc.sync.dma_start(P[:], P_t[b0:b0 + bc])
                _add_dep_helper(dp.ins, dxb.ins, sync=True,
                                reason="serialize input DMAs")
                prev_dma = dp
                loads.append((XA, XB, P))

            # dummy Sqrt so the act-table pass picks the sqrt set for its
            # initial load (otherwise the first Square binds to another set
            # and Sqrt forces a second mid-stream load)
            dum = ps.tile([128, 1], dt, tag="dum")
            nc.gpsimd.memset(dum[:], 1.0)
            nc.scalar.activation(dum[:], dum[:], Act.Sqrt)

            mids = []
            # ---- phase A per chunk: geometry through sqrt (sqrt table) ----
            for c in range(CH):
                XA, XB, P = loads[c]
                XAv = XA[:].rearrange("p (a c s) -> p a c s", a=2, c=3)
                XBv = XB[:].rearrange("p (c s) -> p c s", c=3)
                V = px.tile([128, 9 * R], dt, tag="v")
                Vv = V[:].rearrange("p (v c l) -> p v c l", v=3, c=3)
                # planes: v2 = CA_next - N_next ; v1 = C_prev - N_next ;
                # v3 = CA_prev - C_prev   (XA holds N,CA; XB holds C)
                nc.vector.tensor_sub(Vv[:, 0], XAv[:, 1, :, 1:S],
                                     XAv[:, 0, :, 1:S])
                nc.vector.tensor_sub(Vv[:, 1], XBv[:, :, 0:R],
                                     XAv[:, 0, :, 1:S])
                nc.vector.tensor_sub(Vv[:, 2], XAv[:, 1, :, 0:R],
                                     XBv[:, :, 0:R])

                # W = [v2^2 | v1^2 | v3^2 | v1*v2 | v3*v1] as 5 groups of
                # 3 xyz planes; one pair of strided adds then contracts all
                # five dot products at once.
                W = px.tile([128, 15 * R], dt, tag="w")
                nc.scalar.activation(W[:, 0:9 * R], V[:], Act.Square)
                nc.vector.tensor_mul(W[:, 9 * R:15 * R], V[:, 3 * R:9 * R],
                                     V[:, 0:6 * R])
                Wv = W[:].rearrange("p (g c l) -> p g c l", g=5, c=3)
                DC = ps.tile([128, 5 * R], dt, tag="dc")
                # [d22 | d11 | d33 | c1 | c2]
                DCv = DC[:].rearrange("p (g l) -> p g l", g=5)
                nc.vector.tensor_add(DCv, Wv[:, :, 0], Wv[:, :, 1])
                nc.vector.tensor_add(DCv, DCv, Wv[:, :, 2])

                SQI = ps.tile([128, 4 * R], dt, tag="sqi")  # [s^2 2R | M 2R]
                nc.vector.tensor_mul(SQI[:, 2 * R:3 * R], DC[:, R:2 * R],
                                     DC[:, 0:R])
                nc.vector.tensor_mul(SQI[:, 3 * R:4 * R], DC[:, R:2 * R],
                                     DC[:, 2 * R:3 * R])
                CSQ = ps.tile([128, 2 * R], dt, tag="csq")
                nc.scalar.activation(CSQ[:], DC[:, 3 * R:5 * R], Act.Square)
                nc.vector.tensor_sub(SQI[:, 0:2 * R], SQI[:, 2 * R:4 * R],
                                     CSQ[:])
                # only the s^2 half needs the clamp: M >= 0 by construction
                # and den = s + sqrt(M) >= sqrt(1e-30) > 0 either way
                nc.vector.tensor_scalar_max(SQI[:, 0:2 * R], SQI[:, 0:2 * R],
                                            1e-30)
                SRT = ps.tile([128, 4 * R], dt, tag="srt")  # [s 2R | rtM 2R]
                nc.scalar.activation(SRT[:], SQI[:], Act.Sqrt)

                FB = ps.tile([128, 3 * R], hf, tag="fb")   # [f1 | phi1 | phi2]
                nc.scalar.activation(FB[:, 0:R], DC[:, R:2 * R], Act.Sqrt)

                # den = s + sqrt(M), overwriting s (not needed afterwards)
                nc.vector.tensor_add(SRT[:, 0:2 * R], SRT[:, 0:2 * R],
                                     SRT[:, 2 * R:4 * R])
                REC = ps.tile([128, 2 * R], dt, tag="rec")
                nc.vector.reciprocal_approx_fast(out=REC[:],
                                                 in_=SRT[:, 0:2 * R])
                T = ps.tile([128, 2 * R], hf, tag="t")
                nc.vector.tensor_mul(T[:], DC[:, 3 * R:5 * R], REC[:])
                mids.append((P, T, FB))

            # ---- phase B per chunk: arctan + scoring (trig table) ---------
            with tc.tile_wait_until(1.0):
                for c in range(CH):
                    b0 = c * bc
                    P, T, FB = mids[c]
                    nc.scalar.activation(FB[:, R:3 * R], T[:], Act.Arctan)
                    U = ps.tile([128, 3 * R], hf, tag="u")
                    nc.vector.tensor_mul(U[:], FB[:], P[:, 3 * R:6 * R])
                    nc.vector.tensor_sub(U[:], U[:], P[:, 0:3 * R])
                    nc.scalar.activation(U[:], U[:], Act.Square)
                    nc.vector.tensor_tensor(U[:], U[:], P[:, 6 * R:9 * R],
                                            op=Alu.min)
                    E = ps.tile([128, R], dt, tag="e")
                    nc.vector.tensor_add(E[:], U[:, 0:R], U[:, R:2 * R])
                    nc.vector.tensor_add(E[:], E[:], U[:, 2 * R:3 * R])
                    nc.sync.dma_start(
                        O_t[b0:b0 + bc].rearrange("b c (k l) -> b c k l",
                                                  k=KC),
                        E[:])

    return nc


def _get_program():
    if "nc" not in _PROGRAM_CACHE:
        nc = _build_program()
        nc.finalize()   # Bacc: register allocation / DCE / wait legalization
        _PROGRAM_CACHE["nc"] = nc
    return _PROGRAM_CACHE["nc"]


def _host_prep(atom_description, coords, mean, std, weight):
    ad = np.asarray(atom_description)
    coords = np.asarray(coords, dtype=np.float32)
    b, ch, rs, rn, an = (ad[:, i] for i in range(5))
    valid = (b >= 0) & (b < NB) & (ch >= 0) & (ch < MC) & (rs >= 0) & (rs < MR)

    def scat3(mask):
        A = np.full((NB, MC, MR, 3), PAD, np.float32)
        m = mask & valid
        A[b[m], ch[m], rs[m]] = coords[m]
        return A

    Narr, CAarr, Carr = scat3(an == 0), scat3(an == 1), scat3(an == 2)
    seq = np.full((NB, MC, MR), PAD_I, np.int64)
    m = (an == 1) & valid
    seq[b[m], ch[m], rs[m]] = rn[m]

    todo = ((Narr[:, :, 1:, 0] != PAD) & (Carr[:, :, :-1, 0] != PAD)
            & (CAarr[:, :, 1:, 0] != PAD) & (CAarr[:, :, :-1, 0] != PAD)
            & (seq[:, :, 1:] != PAD_I) & (seq[:, :, :-1] != PAD_I))
    sidx = np.clip(np.where(todo, seq[:, :, 1:], 0), 0, 19)

    w0 = float(np.asarray(weight).reshape(-1)[0])
    s_w = 1.0 - np.tanh(-w0)
    sq = np.sqrt(s_w)
    mu = np.asarray(mean, np.float64)
    sd = np.asarray(std, np.float64)
    q = 1.0 / (sd * np.sqrt(2.0))
    qs = q * sq
    # A = subtractand, B = multiplier for fb=[f1, phi1, phi2], C = clamp.
    # theta1 = pi/2 - 2*phi1 ; theta2 = pi/2 + 2*phi2  (reference's second
    # angle uses N_next-C_prev = -v1; arctan's oddness folds the sign into
    # B2 = -2*q2).
    tab = np.empty((20, 9))
    tab[:, 0] = mu[:, 0] * qs[:, 0]
    tab[:, 1] = (np.pi / 2 - mu[:, 1]) * qs[:, 1]
    tab[:, 2] = (np.pi / 2 - mu[:, 2]) * qs[:, 2]
    tab[:, 3] = qs[:, 0]
    tab[:, 4] = 2.0 * qs[:, 1]
    tab[:, 5] = -2.0 * qs[:, 2]
    tab[:, 6:9] = s_w * np.maximum(np.log(CL * q), 0.0)
    tab = tab.astype(np.float32)

    params = np.zeros((NB, MC, MR, 9), np.float32)
    params[:, :, 1:, :] = tab[sidx] * todo[..., None].astype(np.float32)
    # P row layout per (b,c,k): planar [A0|A1|A2|B0|B1|B2|C0|C1|C2] planes
    # of R, fp16.
    pb = params.reshape(NB, MC, KC, R, 9)
    pblk = np.ascontiguousarray(
        pb.transpose(0, 1, 2, 4, 3)).reshape(NB, MC, KC, 9 * R)
    pblk = pblk.astype(np.float16)

    # G row: planar [atom(N,CA,C)][xyz][slot 0..R]; slot s of block k holds
    # residue k*R + s - 1; content 0.0 where that residue index is < 0.
    G = np.zeros((NB, MC, MR + 1, 3, 3), np.float32)
    G[:, :, 1:, 0] = Narr
    G[:, :, 1:, 1] = CAarr
    G[:, :, 1:, 2] = Carr
    GB = np.empty((NB, MC, KC, 3, 3, S), np.float32)
    for k in range(KC):
        # [b, c, slot, atom, xyz] -> [b, c, atom, xyz, slot]
        GB[:, :, k] = G[:, :, k * R:k * R + S].transpose(0, 1, 3, 4, 2)
    return GB.reshape(NB, MC, KC, 9 * S), pblk


def _install_ntff_hook():
    """The agent image's antenv lacks axon_hooks; synthesize it so
    trace=True can reach the terminal's NRT profiler (dev-only path)."""
    import sys, types
    if "antenv.axon_hooks" in sys.modules:
        return True
    try:
        import antenv
        mod = types.ModuleType("antenv.axon_hooks")
        mod._hook = None

        def set_axon_ntff_profile_hook(h):
            mod._hook = h

        def get_axon_ntff_profile_hook():
            return mod._hook

        mod.set_axon_ntff_profile_hook = set_axon_ntff_profile_hook
        mod.get_axon_ntff_profile_hook = get_axon_ntff_profile_hook
        sys.modules["antenv.axon_hooks"] = mod
        antenv.axon_hooks = mod
        from trn_agent_boot.trn_boot import _ntff_profile_via_ctypes
        mod._hook = _ntff_profile_via_ctypes("/opt/axon/libaxon_pjrt.so")
        return True
    except Exception as e:  # pragma: no cover - profiling is best-effort
        print(f"ntff hook install failed: {e}")
        return False


def kernel(**inputs):
    global LAST_RESULT
    from concourse.bass_utils import run_bass_kernel_spmd
    if TRACE:
        _install_ntff_hook()

    G, pblk = _host_prep(
        inputs["atom_description"], inputs["coords"],
        inputs["mean"], inputs["std"], inputs["weight"])

    nc = _get_program()
    in_maps = [
        {"g": np.ascontiguousarray(G[i * BPC:(i + 1) * BPC]),
         "pr": np.ascontiguousarray(pblk[i * BPC:(i + 1) * BPC])}
        for i in range(NCORES)
    ]
    res = run_bass_kernel_spmd(nc, in_maps, list(range(NCORES)), trace=TRACE)
    LAST_RESULT = res
    e = np.concatenate([res.results[i]["out"] for i in range(NCORES)], axis=0)
    e = e.reshape(NB, MC, MR)
    out = np.repeat(e[..., None], NALT, axis=-1)
    return np.ascontiguousarray(out.astype(np.float32))


# revision 22
# speedup vs baseline: 1.4125x; 1.0838x over previous
"""Trainium2 Bass kernel for nn_BondLenConstrain.

Contract: kernel(**inputs) takes the FULL (unsharded) inputs of
reference.setup_inputs() and returns the full [64, 4, 2048, 2] float32
resiEnergy tensor.  Data-parallel over the batch axis across 8 NeuronCores
(8 batches per core).

Host (numpy, indexing only): scatter atoms into dense residue grids exactly
like the reference, build the `todo` mask, gather the tiny per-residue-type
tables into per-residue coefficient planes (masked pairs get all-zero
coefficients -> device formula returns exactly 0), and broadcast the
(identical) nalt lanes of the output on assembly.

Device math per residue pair (r-1, r), with P=C_{r-1}, Q=N_r, A=CA_r,
B=CA_{r-1}:
    v2 = A-Q, v1 = P-Q, v3 = B-P        (planar [plane][R] layout)
    d22=|v2|^2, d11=|v1|^2, d33=|v3|^2 ; c1 = v1.v2, c2 = v3.v1
    M = d11*d22 (resp. d11*d33), s = sqrt(M - c^2)
    half-angle identity:  angle(v1,v2) = pi/2 - 2*arctan(c1/(s1+sqrt(M1)))
      (argument in [-1,1] automatically; arctan odd -> no sign handling;
       hardware Arctan table domain is ~[-pi/2,pi/2])
    f1 = sqrt(d11)
    U_d = fb_d * B_d - A_d   with fb = [f1, phi1, phi2] and host-baked A,B
    score_d = min(U_d^2, C_d); e = sum_d score_d
A/B/C fold mean/std/weight/todo (masked pairs: A=B=C=0 -> e=0).

Perf structure (v3):
  * input DMAs chained X0 -> P0 -> X1 -> P1 so chunk0's coords get full
    DMA bandwidth instead of fair-sharing with 3 other transfers
  * 1/x via the single-instruction custom-DVE reciprocal_approx_fast
    (nc.vector.reciprocal measured 4us per 512 elems)
  * scoring tail (P coeffs, T, FB, U, Z, ZC) in fp16: DVE 2x_1p mode +
    half the P DMA bytes; rel err ~2.8e-3 on the grading data (gate 2e-2)
  * Square/Sqrt in one act table, Arctan/Square in another; phase A (both
    chunks through sqrt) emitted before phase B under tile_wait_until so
    the Tile scheduler keeps 2 ACT_TABLE_LOADs total
"""

import os
import numpy as np

PAD = -999.0
PAD_I = -999
NB, MC, MR = 64, 4, 2048
NALT = 2
NCORES = 8
BPC = NB // NCORES            # batches per core
CH = int(os.environ.get("BLC_CHUNKS", "2"))  # pipeline chunks per core
KC = 4 * CH                   # blocks per (batch, chain) across full chain
R = MR // KC                  # residues (pairs) per partition
S = R + 1                     # slots per atom plane (halo)
EPS = 1e-12
CL = 1.0 / (EPS * np.sqrt(np.pi))

_PROGRAM_CACHE = {}
LAST_RESULT = None            # BassKernelResults of the last run (for test.py)
TRACE = bool(int(os.environ.get("BLC_TRACE", "0")))


def _build_program():
    import concourse.bass as bass
    import concourse.tile as tile
    from concourse import bacc, mybir
    from concourse.bass import _add_dep_helper

    dt = mybir.dt.float32
    hf = mybir.dt.float16
    Alu = mybir.AluOpType
    Act = mybir.ActivationFunctionType

    nc = bacc.Bacc("TRN2", target_bir_lowering=False, debug=False)

    G_t = nc.declare_dram_parameter("g", [BPC, MC, KC, 9 * S], dt,
                                    isOutput=False)
    P_t = nc.declare_dram_parameter("pr", [BPC, MC, KC, 9 * R], hf,
                                    isOutput=False)
    O_t = nc.declare_dram_parameter("out", [BPC, MC, MR], dt, isOutput=True)

    bc = BPC // CH            # batches per chunk
    bufs = min(CH, 2)

    with tile.TileContext(nc) as tc:
        with (
            tc.tile_pool(name="px", bufs=bufs) as px,
            tc.tile_pool(name="pp", bufs=bufs) as pp,
            tc.tile_pool(name="ps", bufs=bufs) as ps,
        ):
            loads = []
            prev_dma = None
            for c in range(CH):
                b0 = c * bc
                X = px.tile([128, 9 * S], dt, tag="x")
                P = pp.tile([128, 9 * R], hf, tag="p")
                dx = nc.sync.dma_start(X[:], G_t[b0:b0 + bc])
                if prev_dma is not None:
                    _add_dep_helper(dx.ins, prev_dma.ins, sync=True,
                                    reason="serialize input DMAs")
                dp = nc.sync.dma_start(P[:], P_t[b0:b0 + bc])
                _add_dep_helper(dp.ins, dx.ins, sync=True,
                                reason="serialize input DMAs")
                prev_dma = dp
                loads.append((X, P))

            # dummy Sqrt so the act-table pass picks the sqrt set for its
            # initial load (otherwise the first Square binds to another set
            # and Sqrt forces a second mid-stream load)
            dum = ps.tile([128, 1], dt, tag="dum")
            nc.gpsimd.memset(dum[:], 1.0)
            nc.scalar.activation(dum[:], dum[:], Act.Sqrt)

            mids = []
            # ---- phase A per chunk: geometry through sqrt (sqrt table) ----
            for c in range(CH):
                X, P = loads[c]
                Xv = X[:].rearrange("p (a c s) -> p a c s", a=3, c=3)
                V = px.tile([128, 9 * R], dt, tag="v")
                Vv = V[:].rearrange("p (v c l) -> p v c l", v=3, c=3)
                # planes: v2 = CA_next - N_next ; v1 = C_prev - N_next ;
                # v3 = CA_prev - C_prev   (atom order in G: N, CA, C)
                nc.vector.tensor_sub(Vv[:, 0], Xv[:, 1, :, 1:S],
                                     Xv[:, 0, :, 1:S])
                nc.vector.tensor_sub(Vv[:, 1], Xv[:, 2, :, 0:R],
                                     Xv[:, 0, :, 1:S])
                nc.vector.tensor_sub(Vv[:, 2], Xv[:, 1, :, 0:R],
                                     Xv[:, 2, :, 0:R])

                # W = [v2^2 | v1^2 | v3^2 | v1*v2 | v3*v1] as 5 groups of
                # 3 xyz planes; one pair of strided adds then contracts all
                # five dot products at once.
                W = px.tile([128, 15 * R], dt, tag="w")
                nc.scalar.activation(W[:, 0:9 * R], V[:], Act.Square)
                nc.vector.tensor_mul(W[:, 9 * R:15 * R], V[:, 3 * R:9 * R],
                                     V[:, 0:6 * R])
                Wv = W[:].rearrange("p (g c l) -> p g c l", g=5, c=3)
                DC = ps.tile([128, 5 * R], dt, tag="dc")
                # [d22 | d11 | d33 | c1 | c2]
                DCv = DC[:].rearrange("p (g l) -> p g l", g=5)
                nc.vector.tensor_add(DCv, Wv[:, :, 0], Wv[:, :, 1])
                nc.vector.tensor_add(DCv, DCv, Wv[:, :, 2])

                SQI = ps.tile([128, 4 * R], dt, tag="sqi")  # [s^2 2R | M 2R]
                nc.vector.tensor_mul(SQI[:, 2 * R:3 * R], DC[:, R:2 * R],
                                     DC[:, 0:R])
                nc.vector.tensor_mul(SQI[:, 3 * R:4 * R], DC[:, R:2 * R],
                                     DC[:, 2 * R:3 * R])
                CSQ = ps.tile([128, 2 * R], dt, tag="csq")
                nc.scalar.activation(CSQ[:], DC[:, 3 * R:5 * R], Act.Square)
                nc.vector.tensor_sub(SQI[:, 0:2 * R], SQI[:, 2 * R:4 * R],
                                     CSQ[:])
                # only the s^2 half needs the clamp: M >= 0 by construction
                # and den = s + sqrt(M) >= sqrt(1e-30) > 0 either way
                nc.vector.tensor_scalar_max(SQI[:, 0:2 * R], SQI[:, 0:2 * R],
                                            1e-30)
                SRT = ps.tile([128, 4 * R], dt, tag="srt")  # [s 2R | rtM 2R]
                nc.scalar.activation(SRT[:], SQI[:], Act.Sqrt)

                FB = ps.tile([128, 3 * R], hf, tag="fb")   # [f1 | phi1 | phi2]
                nc.scalar.activation(FB[:, 0:R], DC[:, R:2 * R], Act.Sqrt)

                # den = s + sqrt(M), overwriting s (not needed afterwards)
                nc.vector.tensor_add(SRT[:, 0:2 * R], SRT[:, 0:2 * R],
                                     SRT[:, 2 * R:4 * R])
                REC = ps.tile([128, 2 * R], dt, tag="rec")
                nc.vector.reciprocal_approx_fast(out=REC[:],
                                                 in_=SRT[:, 0:2 * R])
                T = ps.tile([128, 2 * R], hf, tag="t")
                nc.vector.tensor_mul(T[:], DC[:, 3 * R:5 * R], REC[:])
                mids.append((P, T, FB))

            # ---- phase B per chunk: arctan + scoring (trig table) ---------
            with tc.tile_wait_until(1.0):
                for c in range(CH):
                    b0 = c * bc
                    P, T, FB = mids[c]
                    nc.scalar.activation(FB[:, R:3 * R], T[:], Act.Arctan)
                    U = ps.tile([128, 3 * R], hf, tag="u")
                    nc.vector.tensor_mul(U[:], FB[:], P[:, 3 * R:6 * R])
                    nc.vector.tensor_sub(U[:], U[:], P[:, 0:3 * R])
                    nc.scalar.activation(U[:], U[:], Act.Square)
                    nc.vector.tensor_tensor(U[:], U[:], P[:, 6 * R:9 * R],
                                            op=Alu.min)
                    E = ps.tile([128, R], dt, tag="e")
                    nc.vector.tensor_add(E[:], U[:, 0:R], U[:, R:2 * R])
                    nc.vector.tensor_add(E[:], E[:], U[:, 2 * R:3 * R])
                    nc.sync.dma_start(
                        O_t[b0:b0 + bc].rearrange("b c (k l) -> b c k l",
                                                  k=KC),
                        E[:])

    return nc


def _get_program():
    if "nc" not in _PROGRAM_CACHE:
        nc = _build_program()
        nc.finalize()   # Bacc: register allocation / DCE / wait legalization
        _PROGRAM_CACHE["nc"] = nc
    return _PROGRAM_CACHE["nc"]


def _host_prep(atom_description, coords, mean, std, weight):
    ad = np.asarray(atom_description)
    coords = np.asarray(coords, dtype=np.float32)
    b, ch, rs, rn, an = (ad[:, i] for i in range(5))
    valid = (b >= 0) & (b < NB) & (ch >= 0) & (ch < MC) & (rs >= 0) & (rs < MR)

    def scat3(mask):
        A = np.full((NB, MC, MR, 3), PAD, np.float32)
        m = mask & valid
        A[b[m], ch[m], rs[m]] = coords[m]
        return A

    Narr, CAarr, Carr = scat3(an == 0), scat3(an == 1), scat3(an == 2)
    seq = np.full((NB, MC, MR), PAD_I, np.int64)
    m = (an == 1) & valid
    seq[b[m], ch[m], rs[m]] = rn[m]

    todo = ((Narr[:, :, 1:, 0] != PAD) & (Carr[:, :, :-1, 0] != PAD)
            & (CAarr[:, :, 1:, 0] != PAD) & (CAarr[:, :, :-1, 0] != PAD)
            & (seq[:, :, 1:] != PAD_I) & (seq[:, :, :-1] != PAD_I))
    sidx = np.clip(np.where(todo, seq[:, :, 1:], 0), 0, 19)

    w0 = float(np.asarray(weight).reshape(-1)[0])
    s_w = 1.0 - np.tanh(-w0)
    sq = np.sqrt(s_w)
    mu = np.asarray(mean, np.float64)
    sd = np.asarray(std, np.float64)
    q = 1.0 / (sd * np.sqrt(2.0))
    qs = q * sq
    # A = subtractand, B = multiplier for fb=[f1, phi1, phi2], C = clamp.
    # theta1 = pi/2 - 2*phi1 ; theta2 = pi/2 + 2*phi2  (reference's second
    # angle uses N_next-C_prev = -v1; arctan's oddness folds the sign into
    # B2 = -2*q2).
    tab = np.empty((20, 9))
    tab[:, 0] = mu[:, 0] * qs[:, 0]
    tab[:, 1] = (np.pi / 2 - mu[:, 1]) * qs[:, 1]
    tab[:, 2] = (np.pi / 2 - mu[:, 2]) * qs[:, 2]
    tab[:, 3] = qs[:, 0]
    tab[:, 4] = 2.0 * qs[:, 1]
    tab[:, 5] = -2.0 * qs[:, 2]
    tab[:, 6:9] = s_w * np.maximum(np.log(CL * q), 0.0)
    tab = tab.astype(np.float32)

    params = np.zeros((NB, MC, MR, 9), np.float32)
    params[:, :, 1:, :] = tab[sidx] * todo[..., None].astype(np.float32)
    # P row layout per (b,c,k): planar [A0|A1|A2|B0|B1|B2|C0|C1|C2] planes
    # of R, fp16.
    pb = params.reshape(NB, MC, KC, R, 9)
    pblk = np.ascontiguousarray(
        pb.transpose(0, 1, 2, 4, 3)).reshape(NB, MC, KC, 9 * R)
    pblk = pblk.astype(np.float16)

    # G row: planar [atom(N,CA,C)][xyz][slot 0..R]; slot s of block k holds
    # residue k*R + s - 1; content 0.0 where that residue index is < 0.
    G = np.zeros((NB, MC, MR + 1, 3, 3), np.float32)
    G[:, :, 1:, 0] = Narr
    G[:, :, 1:, 1] = CAarr
    G[:, :, 1:, 2] = Carr
    GB = np.empty((NB, MC, KC, 3, 3, S), np.float32)
    for k in range(KC):
        # [b, c, slot, atom, xyz] -> [b, c, atom, xyz, slot]
        GB[:, :, k] = G[:, :, k * R:k * R + S].transpose(0, 1, 3, 4, 2)
    return GB.reshape(NB, MC, KC, 9 * S), pblk


def _install_ntff_hook():
    """The agent image's antenv lacks axon_hooks; synthesize it so
    trace=True can reach the terminal's NRT profiler (dev-only path)."""
    import sys, types
    if "antenv.axon_hooks" in sys.modules:
        return True
    try:
        import antenv
        mod = types.ModuleType("antenv.axon_hooks")
        mod._hook = None

        def set_axon_ntff_profile_hook(h):
            mod._hook = h

        def get_axon_ntff_profile_hook():
            return mod._hook

        mod.set_axon_ntff_profile_hook = set_axon_ntff_profile_hook
        mod.get_axon_ntff_profile_hook = get_axon_ntff_profile_hook
        sys.modules["antenv.axon_hooks"] = mod
        antenv.axon_hooks = mod
        from trn_agent_boot.trn_boot import _ntff_profile_via_ctypes
        mod._hook = _ntff_profile_via_ctypes("/opt/axon/libaxon_pjrt.so")
        return True
    except Exception as e:  # pragma: no cover - profiling is best-effort
        print(f"ntff hook install failed: {e}")
        return False


def kernel(**inputs):
    global LAST_RESULT
    from concourse.bass_utils import run_bass_kernel_spmd
    if TRACE:
        _install_ntff_hook()

    G, pblk = _host_prep(
        inputs["atom_description"], inputs["coords"],
        inputs["mean"], inputs["std"], inputs["weight"])

    nc = _get_program()
    in_maps = [
        {"g": np.ascontiguousarray(G[i * BPC:(i + 1) * BPC]),
         "pr": np.ascontiguousarray(pblk[i * BPC:(i + 1) * BPC])}
        for i in range(NCORES)
    ]
    res = run_bass_kernel_spmd(nc, in_maps, list(range(NCORES)), trace=TRACE)
    LAST_RESULT = res
    e = np.concatenate([res.results[i]["out"] for i in range(NCORES)], axis=0)
    e = e.reshape(NB, MC, MR)
    out = np.repeat(e[..., None], NALT, axis=-1)
    return np.ascontiguousarray(out.astype(np.float32))
